# revision 19
# baseline (speedup 1.0000x reference)
"""Trainium2 Bass kernel for a 2-layer GCN decoder (nn_GCNDecoder).

Strategy (8 NeuronCores, SPMD, dst-sharded):
  - Destination nodes sharded 8 ways. Edges (with self-loops) partitioned by
    dst shard, grouped by dst into blocks of <=32 distinct dsts ("slots") x
    <=512 edge lanes (4 chunks of 128).
  - Math: out = A relu(A x W3 + b3) W4 + b4 with A = D^-1/2 (Adj+I) D^-1/2.
    Aggregation commutes with the (linear) feature transforms, so we
    aggregate first in the narrow 64-ch space for BOTH layers:
      program 1:  agg1 = S^T-sum of x_hat[src]  (x_hat = x * dinv, bf16)
                  h
                  hT = relu(W3^T agg1 + b3 (x) sqrtdeg)       [128, W]
                  g = (hT^T W4) scaled by dinv_dst^2 -> g_hat staged rows
      host: halo-exchange gather msg2 = g_hat[src]
      program 2:  agg2 = S-sum of msg2; out = dinv_dst*(agg2 + sqrtdeg (x) b4)
    The dst-side D^-1/2 of layer 1 and the src-side D^-1/2 of layer 2 are
    both folded into the single per-node g_hat scale (dinv^2); biases are
    injected pre-scale as rank-1 PE matmuls (b (x) sqrtdeg) so every
    normalization is either a per-partition Act scale or free in a matmul.
  - Segment-sum aggregation is a PE matmul against a 0/1 selection matrix S
    built on the DVE in 2x mode: S layout [128 lanes, W slots, K chunks]
    (chunk-minor so every DVE operand has a packed last dim).
  - Source features are staged per-edge-lane by the host (halo exchange) in
    bf16; 64 channels per edge for both layers.

Host does: integer packing (vectorized), x_hat prep, bf16 staging of
per-edge rows, the inter-layer gather, and output unpermutation.
"""

import os
import sys
import time
import numpy as np
import ml_dtypes

bf16 = ml_dtypes.bfloat16

# problem constants (spec: nn_GCNDecoder_32959579030036)
N_NODES = 100000
IN_C = 64
HID_C = 128
OUT_C = 64
N_CORES = 8
SHARD = N_NODES // N_CORES   # 12500

W = 32                        # dst slots per block
CPB = 4                       # chunks per block
SLOTS = CPB * 128             # 512 edge lanes per block
GBLK = 16                     # blocks per device loop group
GCH = GBLK * CPB              # 64 chunks per group
GROWS = GBLK * W              # 512 stage rows per group

LAST_HW_EXEC_NS = None

_BASS_READY = False


def _import_bass():
    global _BASS_READY, bacc, tile, mybir, bass_utils
    if _BASS_READY:
        return
    for p in ("/opt/trn_rl_repo", "/opt/pypackages"):
        if os.path.isdir(p) and p not in sys.path:
            sys.path.append(p)
    import concourse.bacc as bacc
    import concourse.tile as tile
    import concourse.mybir as mybir
    from concourse import bass_utils
    _BASS_READY = True


# ----------------------------------------------------------------------------
# host-side packing
# ----------------------------------------------------------------------------

def _pack_core(src, dst, deg):
    """Pack one core's edges (sorted by dst) into blocks of <=W dsts and
    <=SLOTS lanes. Returns per-block lane tables + slot->node map."""
    order = np.argsort(dst, kind="stable")
    src, dst = src[order], dst[order]
    uniq, seg_start, seg_len = np.unique(dst, return_index=True,
                                         return_counts=True)
    assert seg_len.max() <= SLOTS, "node in-degree exceeds block capacity"

    nu = len(uniq)
    block_id = np.empty(nu, np.int64)
    slot_id = np.empty(nu, np.int64)
    lane_start = np.empty(nu, np.int64)
    b = s = lanes = 0
    for i in range(nu):
        c = seg_len[i]
        if s >= W or lanes + c > SLOTS:
            b += 1
            s = lanes = 0
        block_id[i] = b
        slot_id[i] = s
        lane_start[i] = lanes
        s += 1
        lanes += c
    nb = b + 1

    # per-edge expansion (edges already dst-sorted => grouped by uniq)
    iu = np.repeat(np.arange(nu), seg_len)
    within = np.arange(len(dst)) - np.repeat(seg_start, seg_len)
    lane = np.repeat(lane_start, seg_len) + within
    flat = block_id[iu] * SLOTS + lane

    e_src = np.zeros(nb * SLOTS, np.int64)
    e_slot = np.full(nb * SLOTS, -1.0, np.float32)
    e_src[flat] = src
    e_slot[flat] = slot_id[iu]
    slot_node = np.full(nb * W, -1, np.int64)
    slot_node[block_id * W + slot_id] = uniq
    return dict(nb=nb, e_src=e_src.reshape(nb, SLOTS),
                e_slot=e_slot.reshape(nb, SLOTS),
                slot_node=slot_node.reshape(nb, W))


def preprocess(x, edge_index):
    src = np.asarray(edge_index[0], np.int64)
    dst = np.asarray(edge_index[1], np.int64)
    loops = np.arange(N_NODES, dtype=np.int64)
    src_all = np.concatenate([src, loops])
    dst_all = np.concatenate([dst, loops])
    deg = np.bincount(dst_all, minlength=N_NODES).astype(np.float32)
    dinv = 1.0 / np.sqrt(deg)

    shard_of = dst_all // SHARD
    shard_order = np.argsort(shard_of, kind="stable")
    src_all, dst_all = src_all[shard_order], dst_all[shard_order]
    bounds = np.searchsorted(shard_of[shard_order], np.arange(N_CORES + 1))

    cores = []
    for c in range(N_CORES):
        sl = slice(bounds[c], bounds[c + 1])
        cores.append(_pack_core(src_all[sl], dst_all[sl], deg))

    NB = max(c["nb"] for c in cores)
    NB = (NB + 2 * GBLK - 1) // (2 * GBLK) * (2 * GBLK)  # pad to supergroup

    for c in cores:
        pad = NB - c["nb"]
        if pad:
            c["e_src"] = np.concatenate(
                [c["e_src"], np.zeros((pad, SLOTS), np.int64)])
            c["e_slot"] = np.concatenate(
                [c["e_slot"], np.full((pad, SLOTS), -1.0, np.float32)])
            c["slot_node"] = np.concatenate(
                [c["slot_node"], np.full((pad, W), -1, np.int64)])

    # two stage-row maps (partition-major device layouts):
    #  g-stage: node (block b, slot s) -> p*nq + q with q=b//4, p=(b%4)*32+s
    #  out-stage: -> p2*npairs + pr with pr=b//2, p2=(b%2)*32+s
    nq = NB * W // 128
    npairs = NB // 2
    stage_row1 = np.full(N_NODES, -1, np.int64)
    stage_row2 = np.full(N_NODES, -1, np.int64)
    bidx = np.repeat(np.arange(NB), W)
    sidx = np.tile(np.arange(W), NB)
    idx1 = ((bidx % 4) * 32 + sidx) * nq + bidx // 4
    idx2 = ((bidx % 2) * 32 + sidx) * npairs + bidx // 2
    for ci, c in enumerate(cores):
        sn = c["slot_node"].ravel()
        valid = sn >= 0
        stage_row1[sn[valid]] = ci * 128 * nq + idx1[valid]
        stage_row2[sn[valid]] = ci * 64 * npairs + idx2[valid]
    assert (stage_row1 >= 0).all()

    x_hat = (np.asarray(x, np.float32) * dinv[:, None]).astype(bf16)

    NCH = NB * CPB
    out = dict(NB=NB, NCH=NCH, stage_row2=stage_row2, cores=[])
    for c in cores:
        e_src = c["e_src"].reshape(NCH, 128)
        msg1 = np.ascontiguousarray(
            x_hat[e_src].transpose(1, 0, 2)).reshape(128, NCH * IN_C)
        meta_slot = np.ascontiguousarray(
            c["e_slot"].reshape(NCH, 128).T).astype(bf16)      # [128,NCH]
        g2 = stage_row1[e_src]                                  # [NCH,128]
        sn = c["slot_node"].ravel()
        node_deg = np.where(sn >= 0, deg[np.maximum(sn, 0)], np.inf)
        sqrtdeg = np.sqrt(np.where(np.isfinite(node_deg), node_deg, 0.0))
        invdeg = np.where(np.isfinite(node_deg), 1.0 / node_deg, 0.0)
        dinvd = np.where(np.isfinite(node_deg),
                         1.0 / np.sqrt(node_deg), 0.0)
        out["cores"].append(dict(
            msg1=msg1, meta_slot=meta_slot, g2_ind=g2,
            sqrtdeg_row=sqrtdeg.reshape(1, NB * W).astype(bf16),
            invdeg_pp=np.ascontiguousarray(
                invdeg.reshape(nq, 128).T).astype(np.float32),
            dinvd_pp=np.ascontiguousarray(
                dinvd.reshape(NB * W // 64, 64).T).astype(np.float32)))
    return out


# ----------------------------------------------------------------------------
# device programs
# ----------------------------------------------------------------------------

def build_layer1(NB, loop_reps=0, py_reps=1):
    """Program 1: aggregate x_hat messages, apply W3+b3, relu, W4, and the
    combined dinv_dst^2 scale; emit g_hat stage rows (partition-major
    layout [128, nq*64]: node at quad q, lane p lives at [p, q*64:...])."""
    _import_bass()
    NCH = NB * CPB
    ngroups = NB // GBLK
    nsg = ngroups // 2
    nq = NB * W // 128

    nc = bacc.Bacc("TRN2", target_bir_lowering=False, debug=False,
                   num_devices=N_CORES)
    msg_d = nc.dram_tensor("msg", [128, NCH * IN_C], mybir.dt.bfloat16,
                           kind="ExternalInput")
    slot_d = nc.dram_tensor("slot", [128, NCH], mybir.dt.bfloat16,
                            kind="ExternalInput")
    wconst_d = nc.dram_tensor("wconst", [128, W * GCH], mybir.dt.bfloat16,
                              kind="ExternalInput")
    wmat3_d = nc.dram_tensor("wmat3", [IN_C, HID_C], mybir.dt.bfloat16,
                             kind="ExternalInput")
    wmat4_d = nc.dram_tensor("wmat4", [HID_C, OUT_C], mybir.dt.bfloat16,
                             kind="ExternalInput")
    b3_d = nc.dram_tensor("b3row", [1, HID_C], mybir.dt.bfloat16,
                          kind="ExternalInput")
    sqd_d = nc.dram_tensor("sqrtdeg", [1, NB * W], mybir.dt.bfloat16,
                           kind="ExternalInput")
    invdeg_d = nc.dram_tensor("invdeg", [128, nq], mybir.dt.float32,
                              kind="ExternalInput")
    gst_d = nc.dram_tensor("gstage", [128, nq * OUT_C], mybir.dt.bfloat16,
                           kind="ExternalOutput")

    Relu = mybir.ActivationFunctionType.Relu
    Copy = mybir.ActivationFunctionType.Copy
    EQ = mybir.AluOpType.is_equal

    with tile.TileContext(nc) as tc:
        with (
            tc.tile_pool(name="const", bufs=1) as constp,
            tc.tile_pool(name="meta", bufs=1) as metap,
            tc.tile_pool(name="msgs", bufs=4) as msgp,
            tc.tile_pool(name="sel", bufs=2) as selp,
            tc.tile_pool(name="sbuf", bufs=3) as sb,
            tc.tile_pool(name="gout", bufs=2) as gob,
            tc.tile_pool(name="pagg", bufs=2, space="PSUM") as pagg,
            tc.tile_pool(name="ph", bufs=2, space="PSUM") as ph,
            tc.tile_pool(name="pg", bufs=3, space="PSUM") as pg,
        ):
            wconst_t = constp.tile([128, W * GCH], mybir.dt.bfloat16)
            nc.sync.dma_start(wconst_t[:], wconst_d.ap())
            wmat3_t = constp.tile([IN_C, HID_C], mybir.dt.bfloat16)
            nc.sync.dma_start(wmat3_t[:], wmat3_d.ap())
            wmat4_t = constp.tile([HID_C, OUT_C], mybir.dt.bfloat16)
            nc.sync.dma_start(wmat4_t[:], wmat4_d.ap())
            b3_t = constp.tile([1, HID_C], mybir.dt.bfloat16)
            nc.sync.dma_start(b3_t[:], b3_d.ap())
            sqd_t = constp.tile([1, NB * W], mybir.dt.bfloat16)
            nc.sync.dma_start(sqd_t[:], sqd_d.ap())
            invdeg_t = constp.tile([128, nq], mybir.dt.float32)
            nc.sync.dma_start(invdeg_t[:], invdeg_d.ap())
            slot_t = metap.tile([128, NCH], mybir.dt.bfloat16)
            nc.sync.dma_start(slot_t[:], slot_d.ap())

            def body():
                for sg in range(nsg):
                    mt = msgp.tile([128, 2 * GCH * IN_C], mybir.dt.bfloat16,
                                   tag="mt")
                    nc.sync.dma_start(
                        mt[:], msg_d.ap()[:, sg * 2 * GCH * IN_C:
                                          (sg + 1) * 2 * GCH * IN_C])
                    for gh in range(2):
                        g = sg * 2 + gh
                        k0 = g * GCH
                        mof = gh * GCH * IN_C
                        # S[p, w, k] = (w == slot[p, k0+k])  [128, W, GCH]
                        S = selp.tile([128, W * GCH], mybir.dt.bfloat16,
                                      tag="S")
                        slot_b = slot_t[:, k0:k0 + GCH].unsqueeze(
                            1).broadcast_to([128, W, GCH])
                        nc.vector.tensor_tensor(S[:], wconst_t[:], slot_b, EQ)
                        S3 = S[:].rearrange("p (w k) -> p w k", k=GCH)

                        agg = pagg.tile([IN_C, GBLK * W], mybir.dt.float32,
                                        tag="agg")
                        for bl in range(GBLK):
                            for k in range(CPB):
                                kl = bl * CPB + k
                                nc.tensor.matmul(
                                    agg[:, bl * W:(bl + 1) * W],
                                    mt[:, mof + kl * IN_C:
                                       mof + (kl + 1) * IN_C],
                                    S3[:, :, kl],
                                    start=(k == 0), stop=(k == CPB - 1))
                        agg_s = sb.tile([IN_C, GBLK * W], mybir.dt.bfloat16,
                                        tag="aggs")
                        nc.vector.tensor_copy(agg_s[:], agg[:])

                        # whole group's hidden acts in one bank + one relu
                        hp = ph.tile([HID_C, GBLK * W], mybir.dt.float32,
                                     tag="hp")
                        for j in range(GBLK):
                            b = g * GBLK + j
                            nc.tensor.matmul(
                                hp[:, j * W:(j + 1) * W], wmat3_t[:],
                                agg_s[:, j * W:(j + 1) * W],
                                start=True, stop=False)
                            nc.tensor.matmul(
                                hp[:, j * W:(j + 1) * W], b3_t[:],
                                sqd_t[:, b * W:(b + 1) * W],
                                start=False, stop=True)
                        hrelu = sb.tile([HID_C, GBLK * W], mybir.dt.bfloat16,
                                        tag="hrelu")
                        nc.scalar.activation(hrelu[:], hp[:], Relu)

                        gq = gob.tile([128, 4, OUT_C], mybir.dt.bfloat16,
                                      tag="gq")
                        for q in range(4):
                            gp = pg.tile([128, OUT_C], mybir.dt.float32,
                                         tag="gp")
                            for j2 in range(2):
                                nc.tensor.matmul(
                                    gp[j2 * 64:(j2 + 1) * 64, :],
                                    hrelu[:, (q * 2 + j2) * 64:
                                          (q * 2 + j2 + 1) * 64],
                                    wmat4_t[:], start=True, stop=True)
                            nc.scalar.activation(
                                gq[:, q, :], gp[:], Copy,
                                scale=invdeg_t[:, g * 4 + q:g * 4 + q + 1])
                        nc.gpsimd.dma_start(
                            gst_d.ap()[:, g * 4 * OUT_C:(g + 1) * 4 * OUT_C],
                            gq[:])

            if loop_reps:
                with tc.For_i(0, loop_reps, 1):
                    body()
            else:
                for _ in range(py_reps):
                    body()
    nc.compile()
    return nc


def build_layer2(NB, loop_reps=0, py_reps=1):
    """Program 2: aggregate g_hat messages; out = dinvd*(agg + sqrtdeg x b4),
    emitted partition-major [64, npairs*64] f32 (node at pair pr, lane p)."""
    _import_bass()
    NCH = NB * CPB
    ngroups = NB // GBLK
    nsg = ngroups // 2
    npairs = NB // 2

    nc = bacc.Bacc("TRN2", target_bir_lowering=False, debug=False,
                   num_devices=N_CORES)
    msg_d = nc.dram_tensor("msg", [128, NCH * OUT_C], mybir.dt.bfloat16,
                           kind="ExternalInput")
    slot_d = nc.dram_tensor("slot", [128, NCH], mybir.dt.bfloat16,
                            kind="ExternalInput")
    wconst_d = nc.dram_tensor("wconst", [128, W * GCH], mybir.dt.bfloat16,
                              kind="ExternalInput")
    b4_d = nc.dram_tensor("b4row", [1, OUT_C], mybir.dt.bfloat16,
                          kind="ExternalInput")
    sqd_d = nc.dram_tensor("sqrtdeg", [1, NB * W], mybir.dt.bfloat16,
                           kind="ExternalInput")
    dinvd_d = nc.dram_tensor("dinvd", [64, NB // 2], mybir.dt.float32,
                             kind="ExternalInput")
    out_d = nc.dram_tensor("outstage", [64, npairs * OUT_C],
                           mybir.dt.float32, kind="ExternalOutput")

    EQ = mybir.AluOpType.is_equal

    with tile.TileContext(nc) as tc:
        with (
            tc.tile_pool(name="const", bufs=1) as constp,
            tc.tile_pool(name="meta", bufs=1) as metap,
            tc.tile_pool(name="msgs", bufs=4) as msgp,
            tc.tile_pool(name="sel", bufs=2) as selp,
            tc.tile_pool(name="oout", bufs=2) as oob,
            tc.tile_pool(name="pagg", bufs=4, space="PSUM") as pagg,
        ):
            wconst_t = constp.tile([128, W * GCH], mybir.dt.bfloat16)
            nc.sync.dma_start(wconst_t[:], wconst_d.ap())
            b4_t = constp.tile([1, OUT_C], mybir.dt.bfloat16)
            nc.sync.dma_start(b4_t[:], b4_d.ap())
            sqd_t = constp.tile([1, NB * W], mybir.dt.bfloat16)
            nc.sync.dma_start(sqd_t[:], sqd_d.ap())
            dinvd_t = constp.tile([64, NB // 2], mybir.dt.float32)
            nc.sync.dma_start(dinvd_t[:], dinvd_d.ap())
            slot_t = metap.tile([128, NCH], mybir.dt.bfloat16)
            nc.sync.dma_start(slot_t[:], slot_d.ap())

            def body():
                for sg in range(nsg):
                    mt = msgp.tile([128, 2 * GCH * OUT_C], mybir.dt.bfloat16,
                                   tag="mt")
                    nc.sync.dma_start(
                        mt[:], msg_d.ap()[:, sg * 2 * GCH * OUT_C:
                                          (sg + 1) * 2 * GCH * OUT_C])
                    for gh in range(2):
                        g = sg * 2 + gh
                        k0 = g * GCH
                        mof = gh * GCH * OUT_C
                        S = selp.tile([128, W * GCH], mybir.dt.bfloat16,
                                      tag="S")
                        slot_b = slot_t[:, k0:k0 + GCH].unsqueeze(
                            1).broadcast_to([128, W, GCH])
                        nc.vector.tensor_tensor(S[:], wconst_t[:], slot_b, EQ)
                        S3 = S[:].rearrange("p (w k) -> p w k", k=GCH)

                        oq = oob.tile([64, 8, OUT_C], mybir.dt.float32,
                                      tag="oq")
                        for pr in range(8):
                            agg = pagg.tile([64, OUT_C], mybir.dt.float32,
                                            tag="agg")
                            for j in range(2):
                                bl = pr * 2 + j
                                b = g * GBLK + bl
                                for k in range(CPB):
                                    kl = bl * CPB + k
                                    nc.tensor.matmul(
                                        agg[j * W:(j + 1) * W, :],
                                        S3[:, :, kl],
                                        mt[:, mof + kl * OUT_C:
                                           mof + (kl + 1) * OUT_C],
                                        start=(k == 0), stop=False)
                                nc.tensor.matmul(
                                    agg[j * W:(j + 1) * W, :],
                                    sqd_t[:, b * W:(b + 1) * W], b4_t[:],
                                    start=False, stop=True)
                            nc.vector.tensor_scalar(
                                oq[:, pr, :], agg[:],
                                dinvd_t[:, g * 8 + pr:g * 8 + pr + 1], None,
                                mybir.AluOpType.mult)
                        nc.gpsimd.dma_start(
                            out_d.ap()[:, g * 8 * OUT_C:(g + 1) * 8 * OUT_C],
                            oq[:])

            if loop_reps:
                with tc.For_i(0, loop_reps, 1):
                    body()
            else:
                for _ in range(py_reps):
                    body()
    nc.compile()
    return nc


# ----------------------------------------------------------------------------
# full kernel
# ----------------------------------------------------------------------------

def _run(nc, in_maps):
    _import_bass()
    res = bass_utils.run_bass_kernel_spmd(nc, in_maps,
                                          core_ids=list(range(N_CORES)))
    return res.results


def _wconst_np():
    return np.tile(np.repeat(np.arange(W, dtype=np.float32), GCH),
                   (128, 1)).astype(bf16)


def kernel(x, edge_index, W3, b3, W4, b4):
    global LAST_HW_EXEC_NS
    _import_bass()
    prep = preprocess(np.asarray(x, np.float32), np.asarray(edge_index))
    NB, NCH = prep["NB"], prep["NCH"]
    nq = NB * W // 128

    wconst_np = _wconst_np()
    W3_bf = np.asarray(W3, np.float32).astype(bf16)
    W4_bf = np.asarray(W4, np.float32).astype(bf16)
    b3_bf = np.asarray(b3, np.float32).reshape(1, HID_C).astype(bf16)
    b4_bf = np.asarray(b4, np.float32).reshape(1, OUT_C).astype(bf16)

    nc1 = build_layer1(NB)
    in1 = [dict(msg=c["msg1"], slot=c["meta_slot"], wconst=wconst_np,
                wmat3=W3_bf, wmat4=W4_bf, b3row=b3_bf,
                sqrtdeg=c["sqrtdeg_row"], invdeg=c["invdeg_pp"])
           for c in prep["cores"]]
    res1 = _run(nc1, in1)
    # per core: [128, nq*64] -> stage rows [128*nq, 64] at index p*nq+q
    g_hat = np.concatenate([
        np.asarray(r["gstage"]).reshape(128 * nq, OUT_C) for r in res1])

    nc2 = build_layer2(NB)
    in2 = []
    for c in prep["cores"]:
        msg2 = np.ascontiguousarray(
            g_hat[c["g2_ind"]].transpose(1, 0, 2)).reshape(128, NCH * OUT_C)
        in2.append(dict(msg=msg2, slot=c["meta_slot"], wconst=wconst_np,
                        b4row=b4_bf, sqrtdeg=c["sqrtdeg_row"],
                        dinvd=c["dinvd_pp"]))
    res2 = _run(nc2, in2)
    npairs = NB // 2
    outstage = np.concatenate([
        np.asarray(r["outstage"]).reshape(64 * npairs, OUT_C) for r in res2])

    out = outstage[prep["stage_row2"]].astype(np.float32)

    if os.environ.get("KERNEL_BENCH", "0") == "1":
        LAST_HW_EXEC_NS = _bench(NB, in1, in2)
    return out


def _bench(NB, in1, in2, r_lo=16, r_hi=128, rounds=12):
    """Per-rep device time via hardware-loop deltas (loop_reps=r_hi vs r_lo),
    sampled interleaved with a persistent jitted executable and
    device-resident inputs; median of paired diffs rejects drift."""
    out = []
    for builder, ins in ((build_layer1, in1), (build_layer2, in2)):
        r1 = _make_runner(builder(NB, loop_reps=r_lo), ins)
        r2 = _make_runner(builder(NB, loop_reps=r_hi), ins)
        for r in (r1, r2):
            r(); r()
        diffs = []
        for _ in range(rounds):
            t0 = time.perf_counter(); r1(); t1 = time.perf_counter()
            r2(); t2 = time.perf_counter()
            diffs.append(((t2 - t1) - (t1 - t0)) / (r_hi - r_lo))
        out.append(float(np.median(diffs)))
    print(f"[bench] layer1 {out[0]*1e6:.1f} us  layer2 {out[1]*1e6:.1f} us",
          flush=True)
    return (out[0] + out[1]) * 1e9


def _make_runner(nc, in_maps):
    import jax
    import jax.numpy as jnp
    from jax.sharding import Mesh, PartitionSpec
    from jax.experimental.shard_map import shard_map
    import concourse.mybir as mybir
    from concourse import bass2jax
    from concourse.bass2jax import _bass_exec_p, install_neuronx_cc_hook
    install_neuronx_cc_hook()
    n_cores = len(in_maps)
    partition_name = (nc.partition_id_tensor.name
                      if nc.partition_id_tensor else None)
    in_names, out_names, out_avals, zero_outs = [], [], [], []
    for alloc in nc.m.functions[0].allocations:
        if not isinstance(alloc, mybir.MemoryLocationSet):
            continue
        name = alloc.memorylocations[0].name
        if alloc.kind == "ExternalInput":
            if name != partition_name:
                in_names.append(name)
        elif alloc.kind == "ExternalOutput":
            dt = mybir.dt.np(alloc.dtype)
            out_names.append(name)
            out_avals.append(jax.core.ShapedArray(tuple(alloc.tensor_shape),
                                                  dt))
            zero_outs.append(np.zeros(alloc.tensor_shape, dt))

    assert nc.dbg_addr is None
    n_params = len(in_names)
    in_names = in_names + out_names          # donated zero outputs
    if partition_name is not None:
        in_names.append(partition_name)

    def _body(*args):
        operands = list(args)
        if partition_name is not None:
            operands.append(bass2jax.partition_id_tensor())
        outs = _bass_exec_p.bind(
            *operands, out_avals=tuple(out_avals), in_names=tuple(in_names),
            out_names=tuple(out_names), lowering_input_output_aliases=(),
            sim_require_finite=True, sim_require_nnan=True, nc=nc)
        return tuple(outs)

    devices = jax.devices()[:n_cores]
    mesh = Mesh(np.asarray(devices), ("core",))
    n_in = n_params + len(zero_outs)
    donate = tuple(range(n_params, n_params + len(out_names)))
    sharded = jax.jit(shard_map(
        _body, mesh=mesh,
        in_specs=(PartitionSpec("core"),) * n_in,
        out_specs=(PartitionSpec("core"),) * len(out_names),
        check_rep=False), donate_argnums=donate, keep_unused=True)
    concat_in = [np.concatenate([in_maps[c][n] for c in range(n_cores)],
                                axis=0) for n in in_names[:n_params]]
    concat_zero = [np.zeros((n_cores * z.shape[0], *z.shape[1:]), z.dtype)
                   for z in zero_outs]
    dev_in = [jax.device_put(a) for a in concat_in]
    from jax.sharding import NamedSharding
    shardings = [NamedSharding(mesh, PartitionSpec("core"))
                 for _ in concat_zero]
    zeros_fn = jax.jit(
        lambda: tuple(jnp.zeros(z.shape, z.dtype) for z in concat_zero),
        out_shardings=tuple(shardings))

    def run():
        zo = zeros_fn()
        outs = sharded(*dev_in, *zo)
        jax.block_until_ready(outs)
        return outs
    return run


def _bench_calls(runner, n=8, warmup=2):
    for _ in range(warmup):
        runner()
    ts = []
    for _ in range(n):
        t0 = time.perf_counter()
        runner()
        ts.append(time.perf_counter() - t0)
    return ts




# revision 22
# speedup vs baseline: 1.2135x; 1.2135x over previous
"""Trainium2 Bass kernel for a 2-layer GCN decoder (nn_GCNDecoder).

Strategy (8 NeuronCores, SPMD, dst-sharded):
  - Destination nodes sharded 8 ways. Edges (with self-loops) partitioned by
    dst shard, grouped by dst into blocks of <=32 distinct dsts ("slots") x
    <=512 edge lanes (4 chunks of 128).
  - Math: out = A relu(A x W3 + b3) W4 + b4 with A = D^-1/2 (Adj+I) D^-1/2.
    Aggregation commutes with the (linear) feature transforms, so we
    aggregate first in the narrow 64-ch space for BOTH layers:
      program 1:  agg1 = S^T-sum of x_hat[src]  (x_hat = x * dinv, bf16)
                  h
                  hT = relu(W3^T agg1 + b3 (x) sqrtdeg)       [128, W]
                  g = (hT^T W4) scaled by dinv_dst^2 -> g_hat staged rows
      host: halo-exchange gather msg2 = g_hat[src]
      program 2:  agg2 = S-sum of msg2; out = dinv_dst*(agg2 + sqrtdeg (x) b4)
    The dst-side D^-1/2 of layer 1 and the src-side D^-1/2 of layer 2 are
    both folded into the single per-node g_hat scale (dinv^2); biases are
    injected pre-scale as rank-1 PE matmuls (b (x) sqrtdeg) so every
    normalization is either a per-partition Act scale or free in a matmul.
  - Segment-sum aggregation is a PE matmul against a 0/1 selection matrix S
    built on the DVE in 2x mode: S layout [128 lanes, W slots, K chunks]
    (chunk-minor so every DVE operand has a packed last dim).
  - Source features are staged per-edge-lane by the host (halo exchange) in
    bf16; 64 channels per edge for both layers.

Host does: integer packing (vectorized), x_hat prep, bf16 staging of
per-edge rows, the inter-layer gather, and output unpermutation.
"""

import os
import sys
import time
import numpy as np
import ml_dtypes

bf16 = ml_dtypes.bfloat16

# problem constants (spec: nn_GCNDecoder_32959579030036)
N_NODES = 100000
IN_C = 64
HID_C = 128
OUT_C = 64
N_CORES = 8
SHARD = N_NODES // N_CORES   # 12500

W = 32                        # dst slots per block
CPB = 4                       # chunks per block
SLOTS = CPB * 128             # 512 edge lanes per block
GBLK = 16                     # blocks per device loop group
GCH = GBLK * CPB              # 64 chunks per group
GROWS = GBLK * W              # 512 stage rows per group

LAST_HW_EXEC_NS = None

_BASS_READY = False


def _import_bass():
    global _BASS_READY, bacc, tile, mybir, bass_utils
    if _BASS_READY:
        return
    for p in ("/opt/trn_rl_repo", "/opt/pypackages"):
        if os.path.isdir(p) and p not in sys.path:
            sys.path.append(p)
    import concourse.bacc as bacc
    import concourse.tile as tile
    import concourse.mybir as mybir
    from concourse import bass_utils
    _BASS_READY = True


# ----------------------------------------------------------------------------
# host-side packing
# ----------------------------------------------------------------------------

def _pack_core(src, dst, deg):
    """Pack one core's edges (sorted by dst) into blocks of <=W dsts and
    <=SLOTS lanes. Returns per-block lane tables + slot->node map."""
    order = np.argsort(dst, kind="stable")
    src, dst = src[order], dst[order]
    uniq, seg_start, seg_len = np.unique(dst, return_index=True,
                                         return_counts=True)
    assert seg_len.max() <= SLOTS, "node in-degree exceeds block capacity"

    nu = len(uniq)
    block_id = np.empty(nu, np.int64)
    slot_id = np.empty(nu, np.int64)
    lane_start = np.empty(nu, np.int64)
    b = s = lanes = 0
    for i in range(nu):
        c = seg_len[i]
        if s >= W or lanes + c > SLOTS:
            b += 1
            s = lanes = 0
        block_id[i] = b
        slot_id[i] = s
        lane_start[i] = lanes
        s += 1
        lanes += c
    nb = b + 1

    # per-edge expansion (edges already dst-sorted => grouped by uniq)
    iu = np.repeat(np.arange(nu), seg_len)
    within = np.arange(len(dst)) - np.repeat(seg_start, seg_len)
    lane = np.repeat(lane_start, seg_len) + within
    flat = block_id[iu] * SLOTS + lane

    e_src = np.zeros(nb * SLOTS, np.int64)
    e_slot = np.full(nb * SLOTS, -1.0, np.float32)
    e_src[flat] = src
    e_slot[flat] = slot_id[iu]
    slot_node = np.full(nb * W, -1, np.int64)
    slot_node[block_id * W + slot_id] = uniq
    return dict(nb=nb, e_src=e_src.reshape(nb, SLOTS),
                e_slot=e_slot.reshape(nb, SLOTS),
                slot_node=slot_node.reshape(nb, W))


def preprocess(x, edge_index):
    src = np.asarray(edge_index[0], np.int64)
    dst = np.asarray(edge_index[1], np.int64)
    loops = np.arange(N_NODES, dtype=np.int64)
    src_all = np.concatenate([src, loops])
    dst_all = np.concatenate([dst, loops])
    deg = np.bincount(dst_all, minlength=N_NODES).astype(np.float32)
    dinv = 1.0 / np.sqrt(deg)

    shard_of = dst_all // SHARD
    shard_order = np.argsort(shard_of, kind="stable")
    src_all, dst_all = src_all[shard_order], dst_all[shard_order]
    bounds = np.searchsorted(shard_of[shard_order], np.arange(N_CORES + 1))

    cores = []
    for c in range(N_CORES):
        sl = slice(bounds[c], bounds[c + 1])
        cores.append(_pack_core(src_all[sl], dst_all[sl], deg))

    NB = max(c["nb"] for c in cores)
    NB = (NB + 2 * GBLK - 1) // (2 * GBLK) * (2 * GBLK)  # pad to supergroup

    for c in cores:
        pad = NB - c["nb"]
        if pad:
            c["e_src"] = np.concatenate(
                [c["e_src"], np.zeros((pad, SLOTS), np.int64)])
            c["e_slot"] = np.concatenate(
                [c["e_slot"], np.full((pad, SLOTS), -1.0, np.float32)])
            c["slot_node"] = np.concatenate(
                [c["slot_node"], np.full((pad, W), -1, np.int64)])

    # two stage-row maps (partition-major device layouts):
    #  g-stage: node (block b, slot s) -> p*nq + q with q=b//4, p=(b%4)*32+s
    #  out-stage: -> p2*npairs + pr with pr=b//2, p2=(b%2)*32+s
    nq = NB * W // 128
    npairs = NB // 2
    stage_row1 = np.full(N_NODES, -1, np.int64)
    stage_row2 = np.full(N_NODES, -1, np.int64)
    bidx = np.repeat(np.arange(NB), W)
    sidx = np.tile(np.arange(W), NB)
    idx1 = ((bidx % 4) * 32 + sidx) * nq + bidx // 4
    idx2 = ((bidx % 2) * 32 + sidx) * npairs + bidx // 2
    for ci, c in enumerate(cores):
        sn = c["slot_node"].ravel()
        valid = sn >= 0
        stage_row1[sn[valid]] = ci * 128 * nq + idx1[valid]
        stage_row2[sn[valid]] = ci * 64 * npairs + idx2[valid]
    assert (stage_row1 >= 0).all()

    x_hat = (np.asarray(x, np.float32) * dinv[:, None]).astype(bf16)

    NCH = NB * CPB
    out = dict(NB=NB, NCH=NCH, stage_row2=stage_row2, cores=[])
    for c in cores:
        e_src = c["e_src"].reshape(NCH, 128)
        msg1 = np.ascontiguousarray(
            x_hat[e_src].transpose(1, 0, 2)).reshape(128, NCH * IN_C)
        meta_slot = np.ascontiguousarray(
            c["e_slot"].reshape(NCH, 128).T).astype(bf16)      # [128,NCH]
        g2 = stage_row1[e_src]                                  # [NCH,128]
        sn = c["slot_node"].ravel()
        node_deg = np.where(sn >= 0, deg[np.maximum(sn, 0)], np.inf)
        sqrtdeg = np.sqrt(np.where(np.isfinite(node_deg), node_deg, 0.0))
        invdeg = np.where(np.isfinite(node_deg), 1.0 / node_deg, 0.0)
        dinvd = np.where(np.isfinite(node_deg),
                         1.0 / np.sqrt(node_deg), 0.0)
        out["cores"].append(dict(
            msg1=msg1, meta_slot=meta_slot, g2_ind=g2,
            sqrtdeg_row=sqrtdeg.reshape(1, NB * W).astype(bf16),
            invdeg_pp=np.ascontiguousarray(
                invdeg.reshape(nq, 128).T).astype(np.float32),
            dinvd_pp=np.ascontiguousarray(
                dinvd.reshape(NB * W // 64, 64).T).astype(np.float32)))
    return out


# ----------------------------------------------------------------------------
# device programs
# ----------------------------------------------------------------------------

def build_layer1(NB, loop_reps=0, py_reps=1):
    """Program 1: aggregate x_hat messages, apply W3+b3, relu, W4, and the
    combined dinv_dst^2 scale; emit g_hat stage rows (partition-major
    layout [128, nq*64]: node at quad q, lane p lives at [p, q*64:...])."""
    _import_bass()
    NCH = NB * CPB
    ngroups = NB // GBLK
    nsg = ngroups // 2
    nq = NB * W // 128

    nc = bacc.Bacc("TRN2", target_bir_lowering=False, debug=False,
                   num_devices=N_CORES)
    msg_d = nc.dram_tensor("msg", [128, NCH * IN_C], mybir.dt.bfloat16,
                           kind="ExternalInput")
    slot_d = nc.dram_tensor("slot", [128, NCH], mybir.dt.bfloat16,
                            kind="ExternalInput")
    wconst_d = nc.dram_tensor("wconst", [128, W * GCH], mybir.dt.bfloat16,
                              kind="ExternalInput")
    wmat3_d = nc.dram_tensor("wmat3", [IN_C, HID_C], mybir.dt.bfloat16,
                             kind="ExternalInput")
    wmat4_d = nc.dram_tensor("wmat4", [HID_C, OUT_C], mybir.dt.bfloat16,
                             kind="ExternalInput")
    b3_d = nc.dram_tensor("b3row", [1, HID_C], mybir.dt.bfloat16,
                          kind="ExternalInput")
    sqd_d = nc.dram_tensor("sqrtdeg", [1, NB * W], mybir.dt.bfloat16,
                           kind="ExternalInput")
    invdeg_d = nc.dram_tensor("invdeg", [128, nq], mybir.dt.float32,
                              kind="ExternalInput")
    gst_d = nc.dram_tensor("gstage", [128, nq * OUT_C], mybir.dt.bfloat16,
                           kind="ExternalOutput")

    Relu = mybir.ActivationFunctionType.Relu
    Copy = mybir.ActivationFunctionType.Copy
    EQ = mybir.AluOpType.is_equal

    with tile.TileContext(nc) as tc:
        with (
            tc.tile_pool(name="const", bufs=1) as constp,
            tc.tile_pool(name="meta", bufs=1) as metap,
            tc.tile_pool(name="msgs", bufs=4) as msgp,
            tc.tile_pool(name="sel", bufs=2) as selp,
            tc.tile_pool(name="sbuf", bufs=3) as sb,
            tc.tile_pool(name="gout", bufs=2) as gob,
            tc.tile_pool(name="pagg", bufs=2, space="PSUM") as pagg,
            tc.tile_pool(name="ph", bufs=2, space="PSUM") as ph,
            tc.tile_pool(name="pg", bufs=3, space="PSUM") as pg,
        ):
            wconst_t = constp.tile([128, W * GCH], mybir.dt.bfloat16)
            nc.sync.dma_start(wconst_t[:], wconst_d.ap())
            wmat3_t = constp.tile([IN_C, HID_C], mybir.dt.bfloat16)
            nc.sync.dma_start(wmat3_t[:], wmat3_d.ap())
            wmat4_t = constp.tile([HID_C, OUT_C], mybir.dt.bfloat16)
            nc.sync.dma_start(wmat4_t[:], wmat4_d.ap())
            b3_t = constp.tile([1, HID_C], mybir.dt.bfloat16)
            nc.sync.dma_start(b3_t[:], b3_d.ap())
            sqd_t = constp.tile([1, NB * W], mybir.dt.bfloat16)
            nc.sync.dma_start(sqd_t[:], sqd_d.ap())
            invdeg_t = constp.tile([128, nq], mybir.dt.float32)
            nc.sync.dma_start(invdeg_t[:], invdeg_d.ap())
            slot_t = metap.tile([128, NCH], mybir.dt.bfloat16)
            nc.sync.dma_start(slot_t[:], slot_d.ap())

            def body():
                for sg in range(nsg):
                    mt = msgp.tile([128, 2 * GCH * IN_C], mybir.dt.bfloat16,
                                   tag="mt")
                    nc.sync.dma_start(
                        mt[:], msg_d.ap()[:, sg * 2 * GCH * IN_C:
                                          (sg + 1) * 2 * GCH * IN_C])
                    for gh in range(2):
                        g = sg * 2 + gh
                        k0 = g * GCH
                        mof = gh * GCH * IN_C
                        # S[p, w, k] = (w == slot[p, k0+k])  [128, W, GCH]
                        S = selp.tile([128, W * GCH], mybir.dt.bfloat16,
                                      tag="S")
                        slot_b = slot_t[:, k0:k0 + GCH].unsqueeze(
                            1).broadcast_to([128, W, GCH])
                        nc.vector.tensor_tensor(S[:], wconst_t[:], slot_b, EQ)
                        S3 = S[:].rearrange("p (w k) -> p w k", k=GCH)

                        agg = pagg.tile([IN_C, GBLK * W], mybir.dt.float32,
                                        tag="agg")
                        for bl in range(GBLK):
                            for k in range(CPB):
                                kl = bl * CPB + k
                                nc.tensor.matmul(
                                    agg[:, bl * W:(bl + 1) * W],
                                    mt[:, mof + kl * IN_C:
                                       mof + (kl + 1) * IN_C],
                                    S3[:, :, kl],
                                    start=(k == 0), stop=(k == CPB - 1))
                        agg_s = sb.tile([IN_C, GBLK * W], mybir.dt.bfloat16,
                                        tag="aggs")
                        nc.vector.tensor_copy(agg_s[:], agg[:])

                        # whole group's hidden acts in one bank + one relu
                        hp = ph.tile([HID_C, GBLK * W], mybir.dt.float32,
                                     tag="hp")
                        for j in range(GBLK):
                            b = g * GBLK + j
                            nc.tensor.matmul(
                                hp[:, j * W:(j + 1) * W], wmat3_t[:],
                                agg_s[:, j * W:(j + 1) * W],
                                start=True, stop=False)
                            nc.tensor.matmul(
                                hp[:, j * W:(j + 1) * W], b3_t[:],
                                sqd_t[:, b * W:(b + 1) * W],
                                start=False, stop=True)
                        hrelu = sb.tile([HID_C, GBLK * W], mybir.dt.bfloat16,
                                        tag="hrelu")
                        nc.scalar.activation(hrelu[:], hp[:], Relu)

                        gq = gob.tile([128, 4, OUT_C], mybir.dt.bfloat16,
                                      tag="gq")
                        for q in range(4):
                            gp = pg.tile([128, OUT_C], mybir.dt.float32,
                                         tag="gp")
                            for j2 in range(2):
                                nc.tensor.matmul(
                                    gp[j2 * 64:(j2 + 1) * 64, :],
                                    hrelu[:, (q * 2 + j2) * 64:
                                          (q * 2 + j2 + 1) * 64],
                                    wmat4_t[:], start=True, stop=True)
                            nc.scalar.activation(
                                gq[:, q, :], gp[:], Copy,
                                scale=invdeg_t[:, g * 4 + q:g * 4 + q + 1])
                        nc.sync.dma_start(
                            gst_d.ap()[:, g * 4 * OUT_C:(g + 1) * 4 * OUT_C],
                            gq[:])

            if loop_reps:
                with tc.For_i(0, loop_reps, 1):
                    body()
            else:
                for _ in range(py_reps):
                    body()
    nc.compile()
    return nc


def build_layer2(NB, loop_reps=0, py_reps=1):
    """Program 2: aggregate g_hat messages; out = dinvd*(agg + sqrtdeg x b4),
    emitted partition-major [64, npairs*64] f32 (node at pair pr, lane p)."""
    _import_bass()
    NCH = NB * CPB
    ngroups = NB // GBLK
    nsg = ngroups // 2
    npairs = NB // 2

    nc = bacc.Bacc("TRN2", target_bir_lowering=False, debug=False,
                   num_devices=N_CORES)
    msg_d = nc.dram_tensor("msg", [128, NCH * OUT_C], mybir.dt.bfloat16,
                           kind="ExternalInput")
    slot_d = nc.dram_tensor("slot", [128, NCH], mybir.dt.bfloat16,
                            kind="ExternalInput")
    wconst_d = nc.dram_tensor("wconst", [128, W * GCH], mybir.dt.bfloat16,
                              kind="ExternalInput")
    b4_d = nc.dram_tensor("b4row", [1, OUT_C], mybir.dt.bfloat16,
                          kind="ExternalInput")
    sqd_d = nc.dram_tensor("sqrtdeg", [1, NB * W], mybir.dt.bfloat16,
                           kind="ExternalInput")
    dinvd_d = nc.dram_tensor("dinvd", [64, NB // 2], mybir.dt.float32,
                             kind="ExternalInput")
    out_d = nc.dram_tensor("outstage", [64, npairs * OUT_C],
                           mybir.dt.float32, kind="ExternalOutput")

    EQ = mybir.AluOpType.is_equal

    with tile.TileContext(nc) as tc:
        with (
            tc.tile_pool(name="const", bufs=1) as constp,
            tc.tile_pool(name="meta", bufs=1) as metap,
            tc.tile_pool(name="msgs", bufs=4) as msgp,
            tc.tile_pool(name="sel", bufs=2) as selp,
            tc.tile_pool(name="oout", bufs=2) as oob,
            tc.tile_pool(name="pagg", bufs=4, space="PSUM") as pagg,
        ):
            wconst_t = constp.tile([128, W * GCH], mybir.dt.bfloat16)
            nc.sync.dma_start(wconst_t[:], wconst_d.ap())
            b4_t = constp.tile([1, OUT_C], mybir.dt.bfloat16)
            nc.sync.dma_start(b4_t[:], b4_d.ap())
            sqd_t = constp.tile([1, NB * W], mybir.dt.bfloat16)
            nc.sync.dma_start(sqd_t[:], sqd_d.ap())
            dinvd_t = constp.tile([64, NB // 2], mybir.dt.float32)
            nc.sync.dma_start(dinvd_t[:], dinvd_d.ap())
            slot_t = metap.tile([128, NCH], mybir.dt.bfloat16)
            nc.sync.dma_start(slot_t[:], slot_d.ap())

            def body():
                for sg in range(nsg):
                    mt = msgp.tile([128, 2 * GCH * OUT_C], mybir.dt.bfloat16,
                                   tag="mt")
                    nc.sync.dma_start(
                        mt[:], msg_d.ap()[:, sg * 2 * GCH * OUT_C:
                                          (sg + 1) * 2 * GCH * OUT_C])
                    for gh in range(2):
                        g = sg * 2 + gh
                        k0 = g * GCH
                        mof = gh * GCH * OUT_C
                        S = selp.tile([128, W * GCH], mybir.dt.bfloat16,
                                      tag="S")
                        slot_b = slot_t[:, k0:k0 + GCH].unsqueeze(
                            1).broadcast_to([128, W, GCH])
                        nc.vector.tensor_tensor(S[:], wconst_t[:], slot_b, EQ)
                        S3 = S[:].rearrange("p (w k) -> p w k", k=GCH)

                        oq = oob.tile([64, 8, OUT_C], mybir.dt.float32,
                                      tag="oq")
                        for pr in range(8):
                            agg = pagg.tile([64, OUT_C], mybir.dt.float32,
                                            tag="agg")
                            for j in range(2):
                                bl = pr * 2 + j
                                b = g * GBLK + bl
                                for k in range(CPB):
                                    kl = bl * CPB + k
                                    nc.tensor.matmul(
                                        agg[j * W:(j + 1) * W, :],
                                        S3[:, :, kl],
                                        mt[:, mof + kl * OUT_C:
                                           mof + (kl + 1) * OUT_C],
                                        start=(k == 0), stop=False)
                                nc.tensor.matmul(
                                    agg[j * W:(j + 1) * W, :],
                                    sqd_t[:, b * W:(b + 1) * W], b4_t[:],
                                    start=False, stop=True)
                            nc.vector.tensor_scalar(
                                oq[:, pr, :], agg[:],
                                dinvd_t[:, g * 8 + pr:g * 8 + pr + 1], None,
                                mybir.AluOpType.mult)
                        nc.sync.dma_start(
                            out_d.ap()[:, g * 8 * OUT_C:(g + 1) * 8 * OUT_C],
                            oq[:])

            if loop_reps:
                with tc.For_i(0, loop_reps, 1):
                    body()
            else:
                for _ in range(py_reps):
                    body()
    nc.compile()
    return nc


# ----------------------------------------------------------------------------
# full kernel
# ----------------------------------------------------------------------------

def _run(nc, in_maps):
    _import_bass()
    res = bass_utils.run_bass_kernel_spmd(nc, in_maps,
                                          core_ids=list(range(N_CORES)))
    return res.results


def _wconst_np():
    return np.tile(np.repeat(np.arange(W, dtype=np.float32), GCH),
                   (128, 1)).astype(bf16)


def kernel(x, edge_index, W3, b3, W4, b4):
    global LAST_HW_EXEC_NS
    _import_bass()
    prep = preprocess(np.asarray(x, np.float32), np.asarray(edge_index))
    NB, NCH = prep["NB"], prep["NCH"]
    nq = NB * W // 128

    wconst_np = _wconst_np()
    W3_bf = np.asarray(W3, np.float32).astype(bf16)
    W4_bf = np.asarray(W4, np.float32).astype(bf16)
    b3_bf = np.asarray(b3, np.float32).reshape(1, HID_C).astype(bf16)
    b4_bf = np.asarray(b4, np.float32).reshape(1, OUT_C).astype(bf16)

    nc1 = build_layer1(NB)
    in1 = [dict(msg=c["msg1"], slot=c["meta_slot"], wconst=wconst_np,
                wmat3=W3_bf, wmat4=W4_bf, b3row=b3_bf,
                sqrtdeg=c["sqrtdeg_row"], invdeg=c["invdeg_pp"])
           for c in prep["cores"]]
    res1 = _run(nc1, in1)
    # per core: [128, nq*64] -> stage rows [128*nq, 64] at index p*nq+q
    g_hat = np.concatenate([
        np.asarray(r["gstage"]).reshape(128 * nq, OUT_C) for r in res1])

    nc2 = build_layer2(NB)
    in2 = []
    for c in prep["cores"]:
        msg2 = np.ascontiguousarray(
            g_hat[c["g2_ind"]].transpose(1, 0, 2)).reshape(128, NCH * OUT_C)
        in2.append(dict(msg=msg2, slot=c["meta_slot"], wconst=wconst_np,
                        b4row=b4_bf, sqrtdeg=c["sqrtdeg_row"],
                        dinvd=c["dinvd_pp"]))
    res2 = _run(nc2, in2)
    npairs = NB // 2
    outstage = np.concatenate([
        np.asarray(r["outstage"]).reshape(64 * npairs, OUT_C) for r in res2])

    out = outstage[prep["stage_row2"]].astype(np.float32)

    if os.environ.get("KERNEL_BENCH", "0") == "1":
        LAST_HW_EXEC_NS = _bench(NB, in1, in2)
    return out


def _bench(NB, in1, in2, r_lo=16, r_hi=128, rounds=12):
    """Per-rep device time via hardware-loop deltas (loop_reps=r_hi vs r_lo),
    sampled interleaved with a persistent jitted executable and
    device-resident inputs; median of paired diffs rejects drift."""
    out = []
    for builder, ins in ((build_layer1, in1), (build_layer2, in2)):
        r1 = _make_runner(builder(NB, loop_reps=r_lo), ins)
        r2 = _make_runner(builder(NB, loop_reps=r_hi), ins)
        for r in (r1, r2):
            r(); r()
        diffs = []
        for _ in range(rounds):
            t0 = time.perf_counter(); r1(); t1 = time.perf_counter()
            r2(); t2 = time.perf_counter()
            diffs.append(((t2 - t1) - (t1 - t0)) / (r_hi - r_lo))
        out.append(float(np.median(diffs)))
    print(f"[bench] layer1 {out[0]*1e6:.1f} us  layer2 {out[1]*1e6:.1f} us",
          flush=True)
    return (out[0] + out[1]) * 1e9


def _make_runner(nc, in_maps):
    import jax
    import jax.numpy as jnp
    from jax.sharding import Mesh, PartitionSpec
    from jax.experimental.shard_map import shard_map
    import concourse.mybir as mybir
    from concourse import bass2jax
    from concourse.bass2jax import _bass_exec_p, install_neuronx_cc_hook
    install_neuronx_cc_hook()
    n_cores = len(in_maps)
    partition_name = (nc.partition_id_tensor.name
                      if nc.partition_id_tensor else None)
    in_names, out_names, out_avals, zero_outs = [], [], [], []
    for alloc in nc.m.functions[0].allocations:
        if not isinstance(alloc, mybir.MemoryLocationSet):
            continue
        name = alloc.memorylocations[0].name
        if alloc.kind == "ExternalInput":
            if name != partition_name:
                in_names.append(name)
        elif alloc.kind == "ExternalOutput":
            dt = mybir.dt.np(alloc.dtype)
            out_names.append(name)
            out_avals.append(jax.core.ShapedArray(tuple(alloc.tensor_shape),
                                                  dt))
            zero_outs.append(np.zeros(alloc.tensor_shape, dt))

    assert nc.dbg_addr is None
    n_params = len(in_names)
    in_names = in_names + out_names          # donated zero outputs
    if partition_name is not None:
        in_names.append(partition_name)

    def _body(*args):
        operands = list(args)
        if partition_name is not None:
            operands.append(bass2jax.partition_id_tensor())
        outs = _bass_exec_p.bind(
            *operands, out_avals=tuple(out_avals), in_names=tuple(in_names),
            out_names=tuple(out_names), lowering_input_output_aliases=(),
            sim_require_finite=True, sim_require_nnan=True, nc=nc)
        return tuple(outs)

    devices = jax.devices()[:n_cores]
    mesh = Mesh(np.asarray(devices), ("core",))
    n_in = n_params + len(zero_outs)
    donate = tuple(range(n_params, n_params + len(out_names)))
    sharded = jax.jit(shard_map(
        _body, mesh=mesh,
        in_specs=(PartitionSpec("core"),) * n_in,
        out_specs=(PartitionSpec("core"),) * len(out_names),
        check_rep=False), donate_argnums=donate, keep_unused=True)
    concat_in = [np.concatenate([in_maps[c][n] for c in range(n_cores)],
                                axis=0) for n in in_names[:n_params]]
    concat_zero = [np.zeros((n_cores * z.shape[0], *z.shape[1:]), z.dtype)
                   for z in zero_outs]
    dev_in = [jax.device_put(a) for a in concat_in]
    from jax.sharding import NamedSharding
    shardings = [NamedSharding(mesh, PartitionSpec("core"))
                 for _ in concat_zero]
    zeros_fn = jax.jit(
        lambda: tuple(jnp.zeros(z.shape, z.dtype) for z in concat_zero),
        out_shardings=tuple(shardings))

    def run():
        zo = zeros_fn()
        outs = sharded(*dev_in, *zo)
        jax.block_until_ready(outs)
        return outs
    return run


def _bench_calls(runner, n=8, warmup=2):
    for _ in range(warmup):
        runner()
    ts = []
    for _ in range(n):
        t0 = time.perf_counter()
        runner()
        ts.append(time.perf_counter() - t0)
    return ts


